# revision 10
# baseline (speedup 1.0000x reference)
"""Trainium2 Bass kernel for the 3-layer LSTM scan (nn_Net_2095944040841).

Architecture (per core, batch-sharded 512/8 = 64):
  reference: hid = x @ W1.T + b1; 3 chained LSTMCells over T=1024 with the
  original model's quirky state handling (c3 stays 0; cell3 overwrites c2).

Mapping:
  - Layout: gate rows on partitions, batch (64) on the free axis. States are
    stored transposed: h^T [50pad64, 64].
  - All sigmoids computed via tanh: sigma(z) = (1 + tanh(z/2))/2, with the
    1/2 folded into weights, so one ACT op covers all gates of all cells.
  - States are stored scaled: h_hat = 2h, s_hat = 2c; consuming weights are
    pre-halved on the host, so no extra correction ops are needed.
  - Pipeline skew: at tick k, cell1 computes t=k, cell2 t=k-1, cell3 t=k-2.
    The three cells' elementwise work is merged column-wise into single
    ACT/DVE instructions over [*, 192] tiles (blocks: [cell1|cell3|cell2]).
  - Biases ride a constant-1.0 row (row 64) of the h-state tiles through the
    K=65 recurrent matmuls.

Per tick: 12 gate matmuls + 1 output matmul (PE), 3 tanh (ACT),
4 fused mul/add (DVE), 1 copy (GPSIMD), amortized DMAs.
"""

import sys

sys.path.insert(0, "/opt/trn_rl_repo")

import numpy as np

import concourse.bass as bass
import concourse.tile as tile
from concourse import bacc, mybir

HID = 50
IN_DIM = 20
OUT_DIM = 8
B_FULL = 512
T_FULL = 1024
N_CORES = 8

b = 64          # batch per core
Hp = 64         # padded hidden
CH = 64         # x-chunk length in ticks
OB = 8          # output accumulation ticks per DMA

F32 = mybir.dt.float32

# gate row ranges in the reference 4*HID layout: i, f, g, o
GATES = {"i": slice(0, 50), "f": slice(50, 100), "g": slice(100, 150),
         "o": slice(150, 200)}


def _lhsT(wa, wb, bias_a=None, bias_b=None, k_rows=None, with_bias_row=False):
    """Build a [K, 128] (or [K, 8]) stationary lhsT from two [50, K] gate
    weight blocks (columns 0:64 gate A, 64:128 gate B), optional bias row."""
    K = wa.shape[1] if k_rows is None else k_rows
    rows = K + (1 if with_bias_row else 0)
    out = np.zeros((rows, 128), np.float32)
    out[: wa.shape[1], 0:50] = wa.T
    out[: wb.shape[1], 64:114] = wb.T
    if with_bias_row:
        assert bias_a is not None and bias_b is not None
        out[K, 0:50] = bias_a
        out[K, 64:114] = bias_b
    return out


def prep_params(W1, b1, Wih1, Whh1, bih1, bhh1, Wih2, Whh2, bih2, bhh2,
                Wih3, Whh3, bih3, bhh3, W2, b2):
    """Host-side weight transformation. Returns {name: np.float32 array}."""
    W1 = np.asarray(W1, np.float32)
    Wc1 = np.asarray(Wih1, np.float32) @ W1            # [200, 20]
    bc1 = (np.asarray(Wih1, np.float32) @ np.asarray(b1, np.float32)
           + np.asarray(bih1, np.float32) + np.asarray(bhh1, np.float32))
    cells = {
        1: (Wc1, np.asarray(Whh1, np.float32), bc1, 1.0),
        2: (np.asarray(Wih2, np.float32), np.asarray(Whh2, np.float32),
            np.asarray(bih2, np.float32) + np.asarray(bhh2, np.float32), 0.5),
        3: (np.asarray(Wih3, np.float32), np.asarray(Whh3, np.float32),
            np.asarray(bih3, np.float32) + np.asarray(bhh3, np.float32), 0.5),
    }
    out = {}
    for c, (Wx, Wh, bias, in_scale) in cells.items():
        # per-gate scale: 1/2 for sigmoid gates (tanh trick), 1 for g
        gs = {g: (0.5 if g in "ifo" else 1.0) for g in "ifgo"}
        blk = {g: gs[g] * in_scale * Wx[GATES[g]] for g in "ifgo"}   # [50, K]
        blkh = {g: gs[g] * 0.5 * Wh[GATES[g]] for g in "ifgo"}       # [50, 50]
        bb = {g: gs[g] * bias[GATES[g]] for g in "ifgo"}
        Kx = Wx.shape[1]
        # io group: cols 0:64 = i, 64:128 = o ; gf group: g, f
        out[f"w{c}x_io"] = _lhsT(blk["i"], blk["o"], k_rows=Kx)
        out[f"w{c}x_gf"] = _lhsT(blk["g"], blk["f"], k_rows=Kx)
        wh_io = np.zeros((65, 128), np.float32)
        wh_io[0:50, 0:50] = blkh["i"].T
        wh_io[0:50, 64:114] = blkh["o"].T
        wh_io[64, 0:50] = bb["i"]
        wh_io[64, 64:114] = bb["o"]
        out[f"w{c}h_io"] = wh_io
        wh_gf = np.zeros((65, 128), np.float32)
        wh_gf[0:50, 0:50] = blkh["g"].T
        wh_gf[0:50, 64:114] = blkh["f"].T
        wh_gf[64, 0:50] = bb["g"]
        wh_gf[64, 64:114] = bb["f"]
        out[f"w{c}h_gf"] = wh_gf
    w2e = np.zeros((65, OUT_DIM), np.float32)
    w2e[0:50, :] = 0.5 * np.asarray(W2, np.float32).T
    w2e[64, :] = np.asarray(b2, np.float32)
    out["w2e"] = w2e
    # pad x-weights for cells 2/3 to K=64 rows (input is h_hat with junk rows
    # 50:64 -> zero weights there)
    for c in (2, 3):
        for g in ("io", "gf"):
            w = out[f"w{c}x_{g}"]
            wp = np.zeros((64, 128), np.float32)
            wp[:50] = w[:50]
            out[f"w{c}x_{g}"] = wp
    return out


def _seg2(t_ap, width, stride):
    """Two equally-spaced column segments of an AP slice: free shape
    (2, width), segments at col offsets {0, stride}."""
    return bass.AP(tensor=t_ap.tensor, offset=t_ap.offset,
                   ap=[t_ap.ap[0], [stride, 2], [1, width]])


def build_nc(T=T_FULL):
    """Build the Bass module for one core (SPMD across 8)."""
    nc = bacc.Bacc(None, target_bir_lowering=False)
    TB = 3 * b  # 192, merged tile width; blocks [cell1 | cell3 | cell2]
    BLK1, BLK3, BLK2 = 0, b, 2 * b

    xt = nc.dram_tensor("xt", [IN_DIM, T, b], F32, kind="ExternalInput")
    wnames = {}
    for c in (1, 2, 3):
        Kx = IN_DIM if c == 1 else Hp
        for g in ("io", "gf"):
            wnames[f"w{c}x_{g}"] = nc.dram_tensor(
                f"w{c}x_{g}", [Kx, 128], F32, kind="ExternalInput")
            wnames[f"w{c}h_{g}"] = nc.dram_tensor(
                f"w{c}h_{g}", [65, 128], F32, kind="ExternalInput")
    w2e_d = nc.dram_tensor("w2e", [65, OUT_DIM], F32, kind="ExternalInput")
    out_d = nc.dram_tensor("out", [T, OUT_DIM, b], F32, kind="ExternalOutput")

    n_chunks = (T + CH - 1) // CH

    with tile.TileContext(nc) as tc:
        with (
            tc.tile_pool(name="weights", bufs=1) as wp,
            tc.tile_pool(name="state", bufs=1) as sp,
            tc.tile_pool(name="xs", bufs=1) as xp,
            tc.tile_pool(name="work", bufs=3) as wk,
            tc.tile_pool(name="psum", bufs=3, space="PSUM") as pp,
            tc.tile_pool(name="opsum", bufs=1, space="PSUM") as op_pool,
        ):
            # --- load weights ---
            wt = {}
            for name, d in wnames.items():
                t = wp.tile(list(d.shape), F32, name=name, tag=name)
                nc.sync.dma_start(t[:], d[:])
                wt[name] = t
            w2e = wp.tile([65, OUT_DIM], F32)
            nc.sync.dma_start(w2e[:], w2e_d[:])

            # --- persistent state rings ---
            h_ring = [sp.tile([65, TB], F32, name=f"h{i}", tag=f"h{i}") for i in range(2)]
            s_ring = [sp.tile([128, TB], F32, name=f"s{i}", tag=f"s{i}") for i in range(2)]
            for s in range(2):
                nc.vector.memset(h_ring[s][0:64, :], 0.0)
                nc.vector.memset(h_ring[s][64:65, :], 1.0)
                nc.vector.memset(s_ring[s][64:128, :], 0.0)

            # --- x chunks (double buffered) ---
            xs_ring = [xp.tile([IN_DIM, CH, b], F32, name=f"xs{i}", tag=f"xs{i}") for i in range(2)]
            nc.sync.dma_start(xs_ring[0][:], xt[:, 0:CH, :])

            out_ring = [op_pool.tile([OUT_DIM, OB * b], F32, name=f"ob{i}", tag=f"ob{i}") for i in range(2)]

            for k in range(T + 2):
                p, q = k % 2, (k - 1) % 2
                t1 = min(k, T - 1)          # cell1's timestep (clamped)
                c_idx = t1 // CH
                if k % CH == 0 and k // CH == c_idx and c_idx + 1 < n_chunks:
                    nc.sync.dma_start(
                        xs_ring[(c_idx + 1) % 2][:],
                        xt[:, (c_idx + 1) * CH:(c_idx + 2) * CH, :])
                x_sl = xs_ring[c_idx % 2][:, t1 % CH, :]       # [20, b]
                hq, hp_, sq, sk = h_ring[q], h_ring[p], s_ring[q], s_ring[p]

                r1 = pp.tile([128, TB], F32, tag="r1")
                r2 = pp.tile([128, TB], F32, tag="r2")
                for (psum, gg) in ((r1, "io"), (r2, "gf")):
                    # cell1 block
                    nc.tensor.matmul(psum[:, BLK1:BLK1 + b],
                                     wt[f"w1x_{gg}"][:], x_sl,
                                     start=True, stop=False)
                    nc.tensor.matmul(psum[:, BLK1:BLK1 + b],
                                     wt[f"w1h_{gg}"][:], hq[0:65, BLK1:BLK1 + b],
                                     start=False, stop=True)
                    # cell3 block (input = h2_hat, rec = h3_hat)
                    nc.tensor.matmul(psum[:, BLK3:BLK3 + b],
                                     wt[f"w3x_{gg}"][:], hq[0:64, BLK2:BLK2 + b],
                                     start=True, stop=False)
                    nc.tensor.matmul(psum[:, BLK3:BLK3 + b],
                                     wt[f"w3h_{gg}"][:], hq[0:65, BLK3:BLK3 + b],
                                     start=False, stop=True)
                    # cell2 block (input = h1_hat, rec = h2_hat)
                    nc.tensor.matmul(psum[:, BLK2:BLK2 + b],
                                     wt[f"w2x_{gg}"][:], hq[0:64, BLK1:BLK1 + b],
                                     start=True, stop=False)
                    nc.tensor.matmul(psum[:, BLK2:BLK2 + b],
                                     wt[f"w2h_{gg}"][:], hq[0:65, BLK2:BLK2 + b],
                                     start=False, stop=True)

                # ACT: tanh over all gates.  r1s rows 0:64 = Ti, 64:128 = To;
                # r2s rows 0:64 = Tg, 64:128 = Tf
                r1s = wk.tile([128, TB], F32, tag="r1s")
                r2s = wk.tile([128, TB], F32, tag="r2s")
                nc.scalar.activation(r1s[:], r1[:],
                                     mybir.ActivationFunctionType.Tanh)
                nc.scalar.activation(r2s[:], r2[:],
                                     mybir.ActivationFunctionType.Tanh)

                # DVE chain. a1 lives at partition base 64 so cell2's f*c can
                # read its cell3 block directly (same-base rule).
                at1 = wk.tile([128, TB], F32, tag="a1")
                nc.vector.scalar_tensor_tensor(
                    at1[64:128, :], r1s[0:64, :], 1.0, r2s[0:64, :],
                    mybir.AluOpType.add, mybir.AluOpType.mult)
                a2 = wk.tile([128, 2 * b], F32, tag="a2")
                nc.vector.scalar_tensor_tensor(
                    a2[64:128, 0:b], r2s[64:128, 0:b], 1.0, sq[64:128, 0:b],
                    mybir.AluOpType.add, mybir.AluOpType.mult)
                if k == 1:
                    nc.vector.memset(a2[64:128, b:2 * b], 0.0)  # S3(-1) = 0
                else:
                    nc.vector.scalar_tensor_tensor(
                        a2[64:128, b:2 * b], r2s[64:128, 2 * b:3 * b], 1.0,
                        at1[64:128, BLK3:BLK3 + b],
                        mybir.AluOpType.add, mybir.AluOpType.mult)
                # s_new -> sk cols {0:b (c1), 2b:3b (c2)}
                nc.vector.scalar_tensor_tensor(
                    _seg2(sk[64:128, 0:b], b, 2 * b),
                    a2[64:128, 0:2 * b].rearrange("p (s w) -> p s w", s=2),
                    0.5,
                    _seg2(at1[64:128, 0:b], b, 2 * b),
                    mybir.AluOpType.mult, mybir.AluOpType.add)
                # s3_hat col = a1 cell3 block (same-base copy on gpsimd)
                nc.gpsimd.tensor_copy(sk[64:128, BLK3:BLK3 + b],
                                      at1[64:128, BLK3:BLK3 + b])
                # tanh of c states
                tcx = wk.tile([128, TB], F32, tag="tc")
                nc.scalar.activation(tcx[64:128, :], sk[64:128, :],
                                     mybir.ActivationFunctionType.Tanh,
                                     scale=0.5)
                # h_hat = (To + 1) * tanh(c)
                nc.vector.scalar_tensor_tensor(
                    hp_[0:64, :], r1s[64:128, :], 1.0, tcx[64:128, :],
                    mybir.AluOpType.add, mybir.AluOpType.mult)

                if k == 0:
                    nc.vector.memset(h_ring[0][0:64, b:3 * b], 0.0)
                elif k == 1:
                    nc.vector.memset(h_ring[1][0:64, b:2 * b], 0.0)

                if k >= 2:
                    t3 = k - 2
                    oslot = (t3 // OB) % 2
                    ocol = (t3 % OB) * b
                    nc.tensor.matmul(out_ring[oslot][:, ocol:ocol + b],
                                     w2e[:], hp_[0:65, BLK3:BLK3 + b],
                                     start=True, stop=True)
                    if t3 % OB == OB - 1:
                        t0 = t3 - OB + 1
                        ob_sb = wk.tile([OUT_DIM, OB * b], F32, tag="ob_sb")
                        nc.scalar.copy(ob_sb[:], out_ring[oslot][:])
                        nc.sync.dma_start(
                            out_d[t0:t0 + OB, :, :].rearrange(
                                "t p c -> p t c"),
                            ob_sb[:].rearrange(
                                "p (t c) -> p t c", t=OB))
    nc.compile()
    return nc


def kernel(**inputs):
    from concourse.bass_utils import run_bass_kernel_spmd

    x = np.asarray(inputs["x"], np.float32)          # [512, 1024, 20]
    B, T, _ = x.shape
    params = prep_params(**{k: v for k, v in inputs.items() if k != "x"})
    nc = build_nc(T)

    in_maps = []
    for c in range(N_CORES):
        xc = x[c * b:(c + 1) * b]                    # [64, T, 20]
        xtc = np.ascontiguousarray(xc.transpose(2, 1, 0))   # [20, T, 64]
        m = {"xt": xtc}
        m.update(params)
        in_maps.append(m)

    res = run_bass_kernel_spmd(nc, in_maps, core_ids=list(range(N_CORES)))
    out = np.empty((B, T, OUT_DIM), np.float32)
    for c in range(N_CORES):
        # res: [T, 8, b] -> [b, T, 8]
        out[c * b:(c + 1) * b] = res.results[c]["out"].transpose(2, 0, 1)
    return out


# revision 12
# speedup vs baseline: 2.0271x; 2.0271x over previous
"""Trainium2 Bass kernel for the 3-layer LSTM scan (nn_Net_2095944040841).

Architecture (per core, batch-sharded 512/8 = 64):
  reference: hid = x @ W1.T + b1; 3 chained LSTMCells over T=1024 with the
  original model's quirky state handling (c3 stays 0; cell3 overwrites c2).

Mapping:
  - Layout: gate rows on partitions, batch (64) on the free axis. States are
    stored transposed: h^T [50pad64, 64].
  - All sigmoids computed via tanh: sigma(z) = (1 + tanh(z/2))/2, with the
    1/2 folded into weights, so one ACT op covers all gates of all cells.
  - States are stored scaled: h_hat = 2h, s_hat = 2c; consuming weights are
    pre-halved on the host, so no extra correction ops are needed.
  - Pipeline skew: at tick k, cell1 computes t=k, cell2 t=k-1, cell3 t=k-2.
    The three cells' elementwise work is merged column-wise into single
    ACT/DVE instructions over [*, 192] tiles (blocks: [cell1|cell3|cell2]).
  - Biases ride a constant-1.0 row (row 64) of the h-state tiles through the
    K=65 recurrent matmuls.

Per tick: 12 gate matmuls + 1 output matmul (PE), 3 tanh (ACT),
4 fused mul/add (DVE), 1 copy (GPSIMD), amortized DMAs.
"""

import sys

sys.path.insert(0, "/opt/trn_rl_repo")

import numpy as np

import concourse.bass as bass
import concourse.tile as tile
from concourse import bacc, mybir

HID = 50
IN_DIM = 20
OUT_DIM = 8
B_FULL = 512
T_FULL = 1024
N_CORES = 8

b = 64          # batch per core
Hp = 64         # padded hidden
CH = 64         # x-chunk length in ticks
OB = 8          # output accumulation ticks per DMA

F32 = mybir.dt.float32
BF16 = mybir.dt.bfloat16
CDT = BF16          # compute dtype for weights/states/gates
import ml_dtypes
NP_CDT = ml_dtypes.bfloat16

# gate row ranges in the reference 4*HID layout: i, f, g, o
GATES = {"i": slice(0, 50), "f": slice(50, 100), "g": slice(100, 150),
         "o": slice(150, 200)}


def _lhsT(wa, wb, bias_a=None, bias_b=None, k_rows=None, with_bias_row=False):
    """Build a [K, 128] (or [K, 8]) stationary lhsT from two [50, K] gate
    weight blocks (columns 0:64 gate A, 64:128 gate B), optional bias row."""
    K = wa.shape[1] if k_rows is None else k_rows
    rows = K + (1 if with_bias_row else 0)
    out = np.zeros((rows, 128), np.float32)
    out[: wa.shape[1], 0:50] = wa.T
    out[: wb.shape[1], 64:114] = wb.T
    if with_bias_row:
        assert bias_a is not None and bias_b is not None
        out[K, 0:50] = bias_a
        out[K, 64:114] = bias_b
    return out


def prep_params(W1, b1, Wih1, Whh1, bih1, bhh1, Wih2, Whh2, bih2, bhh2,
                Wih3, Whh3, bih3, bhh3, W2, b2):
    """Host-side weight transformation. Returns {name: np.float32 array}."""
    W1 = np.asarray(W1, np.float32)
    Wc1 = np.asarray(Wih1, np.float32) @ W1            # [200, 20]
    bc1 = (np.asarray(Wih1, np.float32) @ np.asarray(b1, np.float32)
           + np.asarray(bih1, np.float32) + np.asarray(bhh1, np.float32))
    cells = {
        1: (Wc1, np.asarray(Whh1, np.float32), bc1, 1.0),
        2: (np.asarray(Wih2, np.float32), np.asarray(Whh2, np.float32),
            np.asarray(bih2, np.float32) + np.asarray(bhh2, np.float32), 0.5),
        3: (np.asarray(Wih3, np.float32), np.asarray(Whh3, np.float32),
            np.asarray(bih3, np.float32) + np.asarray(bhh3, np.float32), 0.5),
    }
    out = {}
    for c, (Wx, Wh, bias, in_scale) in cells.items():
        # per-gate scale: 1/2 for sigmoid gates (tanh trick), 1 for g
        gs = {g: (0.5 if g in "ifo" else 1.0) for g in "ifgo"}
        blk = {g: gs[g] * in_scale * Wx[GATES[g]] for g in "ifgo"}   # [50, K]
        blkh = {g: gs[g] * 0.5 * Wh[GATES[g]] for g in "ifgo"}       # [50, 50]
        bb = {g: gs[g] * bias[GATES[g]] for g in "ifgo"}
        Kx = Wx.shape[1]
        # io group: cols 0:64 = i, 64:128 = o ; gf group: g, f
        out[f"w{c}x_io"] = _lhsT(blk["i"], blk["o"], k_rows=Kx)
        out[f"w{c}x_gf"] = _lhsT(blk["g"], blk["f"], k_rows=Kx)
        wh_io = np.zeros((65, 128), np.float32)
        wh_io[0:50, 0:50] = blkh["i"].T
        wh_io[0:50, 64:114] = blkh["o"].T
        wh_io[64, 0:50] = bb["i"]
        wh_io[64, 64:114] = bb["o"]
        out[f"w{c}h_io"] = wh_io
        wh_gf = np.zeros((65, 128), np.float32)
        wh_gf[0:50, 0:50] = blkh["g"].T
        wh_gf[0:50, 64:114] = blkh["f"].T
        wh_gf[64, 0:50] = bb["g"]
        wh_gf[64, 64:114] = bb["f"]
        out[f"w{c}h_gf"] = wh_gf
    w2e = np.zeros((65, OUT_DIM), np.float32)
    w2e[0:50, :] = 0.5 * np.asarray(W2, np.float32).T
    w2e[64, :] = np.asarray(b2, np.float32)
    out["w2e"] = w2e
    # pad x-weights for cells 2/3 to K=64 rows (input is h_hat with junk rows
    # 50:64 -> zero weights there)
    for c in (2, 3):
        for g in ("io", "gf"):
            w = out[f"w{c}x_{g}"]
            wp = np.zeros((64, 128), np.float32)
            wp[:50] = w[:50]
            out[f"w{c}x_{g}"] = wp
    return out


def _seg2(t_ap, width, stride):
    """Two equally-spaced column segments of an AP slice: free shape
    (2, width), segments at col offsets {0, stride}."""
    return bass.AP(tensor=t_ap.tensor, offset=t_ap.offset,
                   ap=[t_ap.ap[0], [stride, 2], [1, width]])


def build_nc(T=T_FULL):
    """Build the Bass module for one core (SPMD across 8)."""
    nc = bacc.Bacc(None, target_bir_lowering=False)
    TB = 3 * b  # 192, merged tile width; blocks [cell1 | cell3 | cell2]
    BLK1, BLK3, BLK2 = 0, b, 2 * b

    xt = nc.dram_tensor("xt", [IN_DIM, T, b], CDT, kind="ExternalInput")
    wnames = {}
    for c in (1, 2, 3):
        Kx = IN_DIM if c == 1 else Hp
        for g in ("io", "gf"):
            wnames[f"w{c}x_{g}"] = nc.dram_tensor(
                f"w{c}x_{g}", [Kx, 128], CDT, kind="ExternalInput")
            wnames[f"w{c}h_{g}"] = nc.dram_tensor(
                f"w{c}h_{g}", [65, 128], CDT, kind="ExternalInput")
    w2e_d = nc.dram_tensor("w2e", [65, OUT_DIM], CDT, kind="ExternalInput")
    out_d = nc.dram_tensor("out", [T, OUT_DIM, b], F32, kind="ExternalOutput")

    n_chunks = (T + CH - 1) // CH

    with tile.TileContext(nc) as tc:
        with (
            tc.tile_pool(name="weights", bufs=1) as wp,
            tc.tile_pool(name="state", bufs=1) as sp,
            tc.tile_pool(name="xs", bufs=1) as xp,
            tc.tile_pool(name="work", bufs=3) as wk,
            tc.tile_pool(name="psum", bufs=3, space="PSUM") as pp,
            tc.tile_pool(name="opsum", bufs=1, space="PSUM") as op_pool,
        ):
            # --- load weights ---
            wt = {}
            for name, d in wnames.items():
                t = wp.tile(list(d.shape), CDT, name=name, tag=name)
                nc.sync.dma_start(t[:], d[:])
                wt[name] = t
            w2e = wp.tile([65, OUT_DIM], CDT)
            nc.sync.dma_start(w2e[:], w2e_d[:])

            # --- persistent state rings ---
            h_ring = [sp.tile([65, TB], CDT, name=f"h{i}", tag=f"h{i}") for i in range(2)]
            s_ring = [sp.tile([128, TB], CDT, name=f"s{i}", tag=f"s{i}") for i in range(2)]
            for s in range(2):
                nc.vector.memset(h_ring[s][0:64, :], 0.0)
                nc.vector.memset(h_ring[s][64:65, :], 1.0)
                nc.vector.memset(s_ring[s][64:128, :], 0.0)

            # --- x chunks (double buffered) ---
            xs_ring = [xp.tile([IN_DIM, CH, b], CDT, name=f"xs{i}", tag=f"xs{i}") for i in range(2)]
            nc.sync.dma_start(xs_ring[0][:], xt[:, 0:CH, :])

            out_ring = [op_pool.tile([OUT_DIM, OB * b], F32, name=f"ob{i}", tag=f"ob{i}") for i in range(2)]

            for k in range(T + 2):
                p, q = k % 2, (k - 1) % 2
                t1 = min(k, T - 1)          # cell1's timestep (clamped)
                c_idx = t1 // CH
                if k % CH == 0 and k // CH == c_idx and c_idx + 1 < n_chunks:
                    nc.sync.dma_start(
                        xs_ring[(c_idx + 1) % 2][:],
                        xt[:, (c_idx + 1) * CH:(c_idx + 2) * CH, :])
                x_sl = xs_ring[c_idx % 2][:, t1 % CH, :]       # [20, b]
                hq, hp_, sq, sk = h_ring[q], h_ring[p], s_ring[q], s_ring[p]

                r1 = pp.tile([128, TB], F32, tag="r1")
                r2 = pp.tile([128, TB], F32, tag="r2")
                for (psum, gg) in ((r1, "io"), (r2, "gf")):
                    # cell1 block
                    nc.tensor.matmul(psum[:, BLK1:BLK1 + b],
                                     wt[f"w1x_{gg}"][:], x_sl,
                                     start=True, stop=False)
                    nc.tensor.matmul(psum[:, BLK1:BLK1 + b],
                                     wt[f"w1h_{gg}"][:], hq[0:65, BLK1:BLK1 + b],
                                     start=False, stop=True)
                    # cell3 block (input = h2_hat, rec = h3_hat)
                    nc.tensor.matmul(psum[:, BLK3:BLK3 + b],
                                     wt[f"w3x_{gg}"][:], hq[0:64, BLK2:BLK2 + b],
                                     start=True, stop=False)
                    nc.tensor.matmul(psum[:, BLK3:BLK3 + b],
                                     wt[f"w3h_{gg}"][:], hq[0:65, BLK3:BLK3 + b],
                                     start=False, stop=True)
                    # cell2 block (input = h1_hat, rec = h2_hat)
                    nc.tensor.matmul(psum[:, BLK2:BLK2 + b],
                                     wt[f"w2x_{gg}"][:], hq[0:64, BLK1:BLK1 + b],
                                     start=True, stop=False)
                    nc.tensor.matmul(psum[:, BLK2:BLK2 + b],
                                     wt[f"w2h_{gg}"][:], hq[0:65, BLK2:BLK2 + b],
                                     start=False, stop=True)

                # ACT: tanh over all gates.  r1s rows 0:64 = Ti, 64:128 = To;
                # r2s rows 0:64 = Tg, 64:128 = Tf
                r1s = wk.tile([128, TB], CDT, tag="r1s")
                r2s = wk.tile([128, TB], CDT, tag="r2s")
                nc.scalar.activation(r1s[:], r1[:],
                                     mybir.ActivationFunctionType.Tanh)
                nc.scalar.activation(r2s[:], r2[:],
                                     mybir.ActivationFunctionType.Tanh)

                # DVE chain. a1 lives at partition base 64 so cell2's f*c can
                # read its cell3 block directly (same-base rule).
                at1 = wk.tile([128, TB], CDT, tag="a1")
                nc.vector.scalar_tensor_tensor(
                    at1[64:128, :], r1s[0:64, :], 1.0, r2s[0:64, :],
                    mybir.AluOpType.add, mybir.AluOpType.mult)
                a2 = wk.tile([128, 2 * b], CDT, tag="a2")
                nc.vector.scalar_tensor_tensor(
                    a2[64:128, 0:b], r2s[64:128, 0:b], 1.0, sq[64:128, 0:b],
                    mybir.AluOpType.add, mybir.AluOpType.mult)
                if k == 1:
                    nc.vector.memset(a2[64:128, b:2 * b], 0.0)  # S3(-1) = 0
                else:
                    nc.vector.scalar_tensor_tensor(
                        a2[64:128, b:2 * b], r2s[64:128, 2 * b:3 * b], 1.0,
                        at1[64:128, BLK3:BLK3 + b],
                        mybir.AluOpType.add, mybir.AluOpType.mult)
                # s_new -> sk cols {0:b (c1), 2b:3b (c2)}
                nc.vector.scalar_tensor_tensor(
                    _seg2(sk[64:128, 0:b], b, 2 * b),
                    a2[64:128, 0:2 * b].rearrange("p (s w) -> p s w", s=2),
                    0.5,
                    _seg2(at1[64:128, 0:b], b, 2 * b),
                    mybir.AluOpType.mult, mybir.AluOpType.add)
                # s3_hat col = a1 cell3 block (same-base copy on gpsimd)
                nc.gpsimd.tensor_copy(sk[64:128, BLK3:BLK3 + b],
                                      at1[64:128, BLK3:BLK3 + b])
                # tanh of c states
                tcx = wk.tile([128, TB], CDT, tag="tc")
                nc.scalar.activation(tcx[64:128, :], sk[64:128, :],
                                     mybir.ActivationFunctionType.Tanh,
                                     scale=0.5)
                # h_hat = (To + 1) * tanh(c)
                nc.vector.scalar_tensor_tensor(
                    hp_[0:64, :], r1s[64:128, :], 1.0, tcx[64:128, :],
                    mybir.AluOpType.add, mybir.AluOpType.mult)

                if k == 0:
                    nc.vector.memset(h_ring[0][0:64, b:3 * b], 0.0)
                elif k == 1:
                    nc.vector.memset(h_ring[1][0:64, b:2 * b], 0.0)

                if k >= 2:
                    t3 = k - 2
                    oslot = (t3 // OB) % 2
                    ocol = (t3 % OB) * b
                    nc.tensor.matmul(out_ring[oslot][:, ocol:ocol + b],
                                     w2e[:], hp_[0:65, BLK3:BLK3 + b],
                                     start=True, stop=True)
                    if t3 % OB == OB - 1:
                        t0 = t3 - OB + 1
                        ob_sb = wk.tile([OUT_DIM, OB * b], F32, tag="ob_sb")
                        nc.scalar.copy(ob_sb[:], out_ring[oslot][:])
                        nc.sync.dma_start(
                            out_d[t0:t0 + OB, :, :].rearrange(
                                "t p c -> p t c"),
                            ob_sb[:].rearrange(
                                "p (t c) -> p t c", t=OB))
    nc.compile()
    return nc


def make_in_maps(inputs):
    x = np.asarray(inputs["x"], np.float32)          # [512, 1024, 20]
    params = prep_params(**{k: v for k, v in inputs.items() if k != "x"})
    in_maps = []
    for c in range(N_CORES):
        xc = x[c * b:(c + 1) * b]                    # [64, T, 20]
        xtc = np.ascontiguousarray(xc.transpose(2, 1, 0))   # [20, T, 64]
        m = {"xt": xtc.astype(NP_CDT)}
        m.update({k: v.astype(NP_CDT) for k, v in params.items()})
        in_maps.append(m)
    return in_maps


def gather_out(res, B, T):
    out = np.empty((B, T, OUT_DIM), np.float32)
    for c in range(N_CORES):
        out[c * b:(c + 1) * b] = res.results[c]["out"].transpose(2, 0, 1)
    return out


def kernel(**inputs):
    from concourse.bass_utils import run_bass_kernel_spmd

    x = np.asarray(inputs["x"], np.float32)
    B, T, _ = x.shape
    nc = build_nc(T)
    in_maps = make_in_maps(inputs)

    res = run_bass_kernel_spmd(nc, in_maps, core_ids=list(range(N_CORES)))
    return gather_out(res, B, T)
